# revision 4
# baseline (speedup 1.0000x reference)
"""Trainium2 Bass kernel for vq_codebook (Gaussian-RBF softmax codebook lookup).

reference:
    dist_sq[b,i,k] = (x[b,i] - anchors[k])^2
    w = softmax(-|gamma| * dist_sq, axis=k)
    out[b, i*E+e] = sum_k w[b,i,k] * emb[k,e]

Shapes (hardcoded): x [2048,128] f32, anchors [256] f32, emb [256,64] f32,
gamma scalar f32. Output [2048, 8192] f32.

Strategy: data-parallel over batch across 8 cores (256 batches/core,
M = 256*128 = 32768 scalar elements per core).

Per core:
  z[k,m] = -g*x_m^2 + (2g*a_k)*x_m + (-g*a_k^2)   == -g*(x_m-a_k)^2
  computed by PE as a K=3 matmul: lhsT = Wz [3,128] (two k-halves,
  row-tiled at array rows 0 and 32), rhs = F [3, mchunk] with
  F = [x^2; x; 1]. z lands in PSUM fp32 (exact, no broadcast needed).
  ACT: u = Exp(z) -> bf16 SBUF  (the irreducible compute: 8.4M exps/core)
  PE:  out_psum[m, 0:65] = sum_k u[k,m] * [emb|1][k, e]  (u stationary
       bf16 128-col tiles -> FWL; ones column gives softmax denominator)
  DVE: r = 1/s, out = num * r (per-partition tensor_scalar), DMA out.
"""

import sys

sys.path.insert(0, "/opt/trn_rl_repo")

import numpy as np

import concourse.bass as bass
import concourse.bass2jax as bass2jax
import concourse.mybir as mybir
from concourse.bass_utils import run_bass_kernel_spmd
from concourse.tile import TileContext
from concourse.vector_clock import ScopedClock


def _split_multiwait_bir(bir_json: bytes) -> bytes:
    """This walrus build rejects instructions carrying more than one sync
    wait (codegen setupSyncWait: 'Too many sync wait commands'). Rewrite the
    BIR so any instruction with N>1 waits is preceded by N-1 NoOp carrier
    instructions on the same engine, each holding one wait. Sequencers
    process waits in program order, so semantics are unchanged."""
    import orjson

    d = orjson.loads(bir_json)
    n_split = 0
    for fn in d["functions"]:
        for blk in fn["blocks"]:
            new_insts = []
            dirty = False
            for inst in blk["instructions"]:
                si = inst.get("sync_info")
                waits = (si or {}).get("on_wait") or []
                if len(waits) > 1:
                    dirty = True
                    n_split += 1
                    for j, w in enumerate(waits[:-1]):
                        new_insts.append(
                            {
                                "debug": inst.get("debug", 0),
                                "engine": inst["engine"],
                                "ins": [],
                                "name": f"{inst['name']}-sw{j}",
                                "opcode": "NoOp",
                                "outs": [],
                                "sync_info": {"on_update": [], "on_wait": [w]},
                            }
                        )
                    si["on_wait"] = [waits[-1]]
                new_insts.append(inst)
            if dirty:
                blk["instructions"] = new_insts
    return orjson.dumps(d)


_orig_compile_bir_kernel = bass2jax.compile_bir_kernel


def _patched_compile_bir_kernel(bir_json, tmpdir, neff_name="file.neff"):
    return _orig_compile_bir_kernel(
        _split_multiwait_bir(bir_json), tmpdir, neff_name=neff_name
    )


bass2jax.compile_bir_kernel = _patched_compile_bir_kernel

# problem constants (hardcoded per harness contract)
B, INPUT_DIM, K, E = 2048, 128, 256, 64
N_CORES = 8
B_CORE = B // N_CORES          # 256
M = B_CORE * INPUT_DIM         # 32768 scalar x-elements per core
CHUNK = 512                    # m-elements per pipeline step
N_CHUNKS = M // CHUNK          # 64
KH = K // 2                    # 128 (k-half; k on partitions)

F32 = mybir.dt.float32
BF16 = mybir.dt.bfloat16


class PatchedTileContext(TileContext):
    # This walrus build (CoreV3 setupSyncWait) rejects instructions carrying
    # more than 2 sem waits; the stock Tile tail drain attaches the whole
    # global clock to a single Drain. Split the waits across 1-wait drains.
    def _drain_and_barrier(self, tick_clock, wait_clock):
        drain_inst = self.nc.sync.drain()
        wait_clock.add_sem_waits(
            drain_inst.ins, ScopedClock({None: tick_clock.global_clock})
        )
        si = drain_inst.ins.sync_info
        if si is not None and len(si.on_wait) > 1:
            waits = list(si.on_wait)
            drain_inst.ins.sync_info = mybir.SyncInfo(
                on_wait=waits[:1], on_update=list(si.on_update)
            )
            for w in waits[1:]:
                d2 = self.nc.sync.drain()
                d2.ins.sync_info = mybir.SyncInfo(on_wait=[w], on_update=[])

        self.nc.all_engine_barrier()
        assert self.sems is not None
        popped = self.nc._tile_sem_poison_stack.pop()
        assert popped is self._sem_poison
        self.nc.clear_and_free_semaphores(list(self.sems.allocated().values()))
        self.nc.all_engine_barrier()


def _build_program():
    nc = bass.Bass()
    feats_d = nc.declare_dram_parameter("feats", [3, M], F32, isOutput=False)
    wz_d = nc.declare_dram_parameter("wz", [6, KH], F32, isOutput=False)
    remb_d = nc.declare_dram_parameter("remb", [KH, 2 * (E + 1)], BF16, isOutput=False)
    out_d = nc.declare_dram_parameter("outp", [M, E], F32, isOutput=True)

    EW = E + 1  # 65: emb columns + ones column

    with PatchedTileContext(nc) as tc:
        with (
            tc.tile_pool(name="const", bufs=1) as const_pool,
            tc.tile_pool(name="upool", bufs=3) as upool,
            tc.tile_pool(name="opool", bufs=3) as opool,
            tc.tile_pool(name="rpool", bufs=3) as rpool,
            tc.tile_pool(name="pz", bufs=2, space="PSUM") as pz_pool,
            tc.tile_pool(name="po", bufs=3, space="PSUM") as po_pool,
        ):
            # constants
            feats = const_pool.tile([35, M], F32)
            nc.sync.dma_start(out=feats[0:3, :], in_=feats_d[:, :])
            nc.sync.dma_start(out=feats[32:35, :], in_=feats_d[:, :])
            wz = const_pool.tile([35, KH], F32)
            nc.sync.dma_start(out=wz[0:3, :], in_=wz_d[0:3, :])
            nc.sync.dma_start(out=wz[32:35, :], in_=wz_d[3:6, :])
            remb = const_pool.tile([KH, 2 * EW], BF16)
            nc.sync.dma_start(out=remb[:, :], in_=remb_d[:, :])

            out_r = out_d.rearrange("(c t p) e -> c p t e", p=128, t=4)

            for c in range(N_CHUNKS):
                lo = c * CHUNK
                hi = lo + CHUNK

                # z[k, m] for both k-halves, row-tiled (rows 0-2 / 32-34)
                psum_z = pz_pool.tile([128, 2 * CHUNK], F32)
                nc.tensor.matmul(
                    psum_z[:, 0:CHUNK],
                    wz[0:3, :],
                    feats[0:3, lo:hi],
                    start=True,
                    stop=True,
                )
                nc.tensor.matmul(
                    psum_z[:, CHUNK : 2 * CHUNK],
                    wz[32:35, :],
                    feats[32:35, lo:hi],
                    start=True,
                    stop=True,
                )

                # u = exp(z), bf16
                u_sb = upool.tile([128, 2 * CHUNK], BF16)
                nc.scalar.activation(
                    u_sb[:, :], psum_z[:, :], mybir.ActivationFunctionType.Exp
                )

                # out_psum[m, e] = sum_k u[k,m] * remb[k,e], 4 m-tiles of 128
                psum_o = po_pool.tile([128, 4 * EW], F32)
                for t in range(4):
                    nc.tensor.matmul(
                        psum_o[:, t * EW : (t + 1) * EW],
                        u_sb[:, t * 128 : (t + 1) * 128],
                        remb[:, 0:EW],
                        start=True,
                        stop=False,
                    )
                    nc.tensor.matmul(
                        psum_o[:, t * EW : (t + 1) * EW],
                        u_sb[:, CHUNK + t * 128 : CHUNK + (t + 1) * 128],
                        remb[:, EW : 2 * EW],
                        start=False,
                        stop=True,
                    )

                # normalize: r = 1/s (s = ones-column), out = num * r
                po_3d = psum_o.rearrange("p (t w) -> p t w", w=EW)
                r_sb = rpool.tile([128, 4], F32)
                nc.vector.reciprocal(r_sb[:, :], po_3d[:, :, E])
                out_sb = opool.tile([128, 4 * E], F32)
                for t in range(4):
                    nc.vector.tensor_scalar(
                        out_sb[:, t * E : (t + 1) * E],
                        po_3d[:, t, 0:E],
                        r_sb[:, t : t + 1],
                        None,
                        mybir.AluOpType.mult,
                    )
                nc.sync.dma_start(
                    out=out_r[c],
                    in_=out_sb.rearrange("p (t e) -> p t e", e=E),
                )

    return nc


_NC_CACHE = None


def _get_program():
    global _NC_CACHE
    if _NC_CACHE is None:
        _NC_CACHE = _build_program()
    return _NC_CACHE


def _prep_core_inputs(x_shard, anchors, embeddings, gamma):
    g = float(np.abs(np.float32(gamma)))
    xf = np.ascontiguousarray(x_shard, dtype=np.float32).reshape(-1)  # [M]
    feats = np.empty((3, M), dtype=np.float32)
    feats[0] = xf * xf
    feats[1] = xf
    feats[2] = 1.0
    a = np.asarray(anchors, dtype=np.float32)
    wz = np.empty((6, KH), dtype=np.float32)
    for h in range(2):
        ak = a[h * KH : (h + 1) * KH]
        wz[3 * h + 0] = np.float32(-g)
        wz[3 * h + 1] = np.float32(2.0 * g) * ak
        wz[3 * h + 2] = np.float32(-g) * (ak * ak)
    emb = np.asarray(embeddings, dtype=np.float32)
    import ml_dtypes

    EW = E + 1
    remb = np.zeros((KH, 2 * EW), dtype=ml_dtypes.bfloat16)
    for h in range(2):
        remb[:, h * EW : h * EW + E] = emb[h * KH : (h + 1) * KH, :].astype(
            ml_dtypes.bfloat16
        )
        remb[:, h * EW + E] = np.array(1.0, dtype=ml_dtypes.bfloat16)
    return {"feats": feats, "wz": wz, "remb": remb}


def kernel(x, anchors, embeddings, gamma):
    nc = _get_program()
    in_maps = []
    for core in range(N_CORES):
        x_shard = x[core * B_CORE : (core + 1) * B_CORE]
        in_maps.append(_prep_core_inputs(x_shard, anchors, embeddings, gamma))
    res = run_bass_kernel_spmd(nc, in_maps, list(range(N_CORES)))
    out = np.empty((B, INPUT_DIM * E), dtype=np.float32)
    for core in range(N_CORES):
        out[core * B_CORE : (core + 1) * B_CORE] = (
            res.results[core]["outp"].reshape(B_CORE, INPUT_DIM * E)
        )
    return out


# revision 8
# speedup vs baseline: 1.2518x; 1.2518x over previous
"""Trainium2 Bass kernel for vq_codebook (Gaussian-RBF softmax codebook lookup).

reference:
    dist_sq[b,i,k] = (x[b,i] - anchors[k])^2
    w = softmax(-|gamma| * dist_sq, axis=k)
    out[b, i*E+e] = sum_k w[b,i,k] * emb[k,e]

Shapes (hardcoded): x [2048,128] f32, anchors [256] f32, emb [256,64] f32,
gamma scalar f32. Output [2048, 8192] f32.

Strategy: data-parallel over batch across 8 cores (256 batches/core,
M = 256*128 = 32768 scalar elements per core).

Per core:
  z[k,m] = -g*x_m^2 + (2g*a_k)*x_m + (-g*a_k^2)   == -g*(x_m-a_k)^2
  computed by PE as a K=3 matmul: lhsT = Wz [3,128] (two k-halves,
  row-tiled at array rows 0 and 32), rhs = F [3, mchunk] with
  F = [x^2; x; 1]. z lands in PSUM fp32 (exact, no broadcast needed).
  ACT: u = Exp(z) -> bf16 SBUF  (the irreducible compute: 8.4M exps/core)
  PE:  out_psum[m, 0:65] = sum_k u[k,m] * [emb|1][k, e]  (u stationary
       bf16 128-col tiles -> FWL; ones column gives softmax denominator)
  DVE: r = 1/s, out = num * r (per-partition tensor_scalar), DMA out.
"""

import sys

sys.path.insert(0, "/opt/trn_rl_repo")

import numpy as np

import concourse.bass as bass
import concourse.bass2jax as bass2jax
import concourse.mybir as mybir
from concourse.bass_utils import run_bass_kernel_spmd
from concourse.tile import TileContext
from concourse.vector_clock import ScopedClock


def _split_multiwait_bir(bir_json: bytes) -> bytes:
    """This walrus build rejects instructions carrying more than one sync
    wait (codegen setupSyncWait: 'Too many sync wait commands'). Rewrite the
    BIR so any instruction with N>1 waits is preceded by N-1 NoOp carrier
    instructions on the same engine, each holding one wait. Sequencers
    process waits in program order, so semantics are unchanged."""
    import orjson

    d = orjson.loads(bir_json)
    n_split = 0
    for fn in d["functions"]:
        for blk in fn["blocks"]:
            new_insts = []
            dirty = False
            for inst in blk["instructions"]:
                si = inst.get("sync_info")
                waits = (si or {}).get("on_wait") or []
                if len(waits) > 1:
                    dirty = True
                    n_split += 1
                    for j, w in enumerate(waits[:-1]):
                        new_insts.append(
                            {
                                "debug": inst.get("debug", 0),
                                "engine": inst["engine"],
                                "ins": [],
                                "name": f"{inst['name']}-sw{j}",
                                "opcode": "NoOp",
                                "outs": [],
                                "sync_info": {"on_update": [], "on_wait": [w]},
                            }
                        )
                    si["on_wait"] = [waits[-1]]
                new_insts.append(inst)
            if dirty:
                blk["instructions"] = new_insts
    return orjson.dumps(d)


_orig_compile_bir_kernel = bass2jax.compile_bir_kernel


def _patched_compile_bir_kernel(bir_json, tmpdir, neff_name="file.neff"):
    return _orig_compile_bir_kernel(
        _split_multiwait_bir(bir_json), tmpdir, neff_name=neff_name
    )


bass2jax.compile_bir_kernel = _patched_compile_bir_kernel

# problem constants (hardcoded per harness contract)
B, INPUT_DIM, K, E = 2048, 128, 256, 64
N_CORES = 8
B_CORE = B // N_CORES          # 256
M = B_CORE * INPUT_DIM         # 32768 scalar x-elements per core
CHUNK = 512                    # m-elements per pipeline step
N_CHUNKS = M // CHUNK          # 64
KH = K // 2                    # 128 (k-half; k on partitions)

F32 = mybir.dt.float32
BF16 = mybir.dt.bfloat16


class PatchedTileContext(TileContext):
    # This walrus build (CoreV3 setupSyncWait) rejects instructions carrying
    # more than 2 sem waits; the stock Tile tail drain attaches the whole
    # global clock to a single Drain. Split the waits across 1-wait drains.
    def _drain_and_barrier(self, tick_clock, wait_clock):
        drain_inst = self.nc.sync.drain()
        wait_clock.add_sem_waits(
            drain_inst.ins, ScopedClock({None: tick_clock.global_clock})
        )
        si = drain_inst.ins.sync_info
        if si is not None and len(si.on_wait) > 1:
            waits = list(si.on_wait)
            drain_inst.ins.sync_info = mybir.SyncInfo(
                on_wait=waits[:1], on_update=list(si.on_update)
            )
            for w in waits[1:]:
                d2 = self.nc.sync.drain()
                d2.ins.sync_info = mybir.SyncInfo(on_wait=[w], on_update=[])

        self.nc.all_engine_barrier()
        assert self.sems is not None
        popped = self.nc._tile_sem_poison_stack.pop()
        assert popped is self._sem_poison
        self.nc.clear_and_free_semaphores(list(self.sems.allocated().values()))
        self.nc.all_engine_barrier()


def _build_program():
    nc = bass.Bass()
    feats_d = nc.declare_dram_parameter("feats", [3, M], F32, isOutput=False)
    wz_d = nc.declare_dram_parameter("wz", [6, KH], F32, isOutput=False)
    remb_d = nc.declare_dram_parameter("remb", [KH, 2 * (E + 1)], BF16, isOutput=False)
    out_d = nc.declare_dram_parameter("outp", [M, E], F32, isOutput=True)

    EW = E + 1  # 65: emb columns + ones column

    with PatchedTileContext(nc) as tc:
        with (
            tc.tile_pool(name="const", bufs=1) as const_pool,
            tc.tile_pool(name="upool", bufs=3) as upool,
            tc.tile_pool(name="opool", bufs=3) as opool,
            tc.tile_pool(name="rpool", bufs=3) as rpool,
            tc.tile_pool(name="pz", bufs=2, space="PSUM") as pz_pool,
            tc.tile_pool(name="po", bufs=3, space="PSUM") as po_pool,
        ):
            # constants
            feats = const_pool.tile([35, M], F32)
            nc.sync.dma_start(out=feats[0:3, :], in_=feats_d[:, :])
            nc.sync.dma_start(out=feats[32:35, :], in_=feats_d[:, :])
            wz = const_pool.tile([35, KH], F32)
            nc.sync.dma_start(out=wz[0:3, :], in_=wz_d[0:3, :])
            nc.sync.dma_start(out=wz[32:35, :], in_=wz_d[3:6, :])
            remb = const_pool.tile([KH, 2 * EW], BF16)
            nc.sync.dma_start(out=remb[:, :], in_=remb_d[:, :])

            # Host permutes feats columns so that within chunk c, SBUF column
            # j = t*128 + p computes m = c*512 + 4*p + t. Then out_sb
            # [p, t*64+e] is exactly DRAM offset (c*512 + 4p + t)*64 + e:
            # one fully contiguous 128 KiB DMA per chunk.
            out_r = out_d[:, :].rearrange("(c p w) e -> c p (w e)", p=128, w=4)

            for c in range(N_CHUNKS):
                lo = c * CHUNK
                hi = lo + CHUNK

                # z[k, m] for both k-halves, row-tiled (rows 0-2 / 32-34)
                psum_z = pz_pool.tile([128, 2 * CHUNK], F32)
                nc.tensor.matmul(
                    psum_z[:, 0:CHUNK],
                    wz[0:3, :],
                    feats[0:3, lo:hi],
                    start=True,
                    stop=True,
                )
                nc.tensor.matmul(
                    psum_z[:, CHUNK : 2 * CHUNK],
                    wz[32:35, :],
                    feats[32:35, lo:hi],
                    start=True,
                    stop=True,
                )

                # u = exp(z), bf16
                u_sb = upool.tile([128, 2 * CHUNK], BF16)
                nc.scalar.activation(
                    u_sb[:, :], psum_z[:, :], mybir.ActivationFunctionType.Exp
                )

                # out_psum[m, e] = sum_k u[k,m] * remb[k,e], 4 m-tiles of 128
                psum_o = po_pool.tile([128, 4 * EW], F32)
                for t in range(4):
                    nc.tensor.matmul(
                        psum_o[:, t * EW : (t + 1) * EW],
                        u_sb[:, t * 128 : (t + 1) * 128],
                        remb[:, 0:EW],
                        start=True,
                        stop=False,
                    )
                    nc.tensor.matmul(
                        psum_o[:, t * EW : (t + 1) * EW],
                        u_sb[:, CHUNK + t * 128 : CHUNK + (t + 1) * 128],
                        remb[:, EW : 2 * EW],
                        start=False,
                        stop=True,
                    )

                # normalize: r = 1/s (s = ones-column), out = num * r
                po_3d = psum_o.rearrange("p (t w) -> p t w", w=EW)
                r_sb = rpool.tile([128, 4], F32)
                nc.vector.reciprocal(r_sb[:, :], po_3d[:, :, E])
                out_sb = opool.tile([128, 4 * E], F32)
                for t in range(4):
                    nc.vector.tensor_scalar(
                        out_sb[:, t * E : (t + 1) * E],
                        po_3d[:, t, 0:E],
                        r_sb[:, t : t + 1],
                        None,
                        mybir.AluOpType.mult,
                    )
                nc.sync.dma_start(out=out_r[c], in_=out_sb[:, :])

    return nc


_NC_CACHE = None


def _get_program():
    global _NC_CACHE
    if _NC_CACHE is None:
        _NC_CACHE = _build_program()
    return _NC_CACHE


def _feats_perm():
    # column j = c*512 + t*128 + p of the on-device feats tensor must carry
    # element m = c*512 + 4*p + t (see out_r comment in _build_program)
    j = np.arange(M)
    c, r = j // CHUNK, j % CHUNK
    t, p = r // 128, r % 128
    return c * CHUNK + 4 * p + t


_PERM = None


def _prep_core_inputs(x_shard, anchors, embeddings, gamma):
    global _PERM
    if _PERM is None:
        _PERM = _feats_perm()
    g = float(np.abs(np.float32(gamma)))
    xf = np.ascontiguousarray(x_shard, dtype=np.float32).reshape(-1)[_PERM]  # [M]
    feats = np.empty((3, M), dtype=np.float32)
    feats[0] = xf * xf
    feats[1] = xf
    feats[2] = 1.0
    a = np.asarray(anchors, dtype=np.float32)
    wz = np.empty((6, KH), dtype=np.float32)
    for h in range(2):
        ak = a[h * KH : (h + 1) * KH]
        wz[3 * h + 0] = np.float32(-g)
        wz[3 * h + 1] = np.float32(2.0 * g) * ak
        wz[3 * h + 2] = np.float32(-g) * (ak * ak)
    emb = np.asarray(embeddings, dtype=np.float32)
    import ml_dtypes

    EW = E + 1
    remb = np.zeros((KH, 2 * EW), dtype=ml_dtypes.bfloat16)
    for h in range(2):
        remb[:, h * EW : h * EW + E] = emb[h * KH : (h + 1) * KH, :].astype(
            ml_dtypes.bfloat16
        )
        remb[:, h * EW + E] = np.array(1.0, dtype=ml_dtypes.bfloat16)
    return {"feats": feats, "wz": wz, "remb": remb}


def kernel(x, anchors, embeddings, gamma):
    nc = _get_program()
    in_maps = []
    for core in range(N_CORES):
        x_shard = x[core * B_CORE : (core + 1) * B_CORE]
        in_maps.append(_prep_core_inputs(x_shard, anchors, embeddings, gamma))
    res = run_bass_kernel_spmd(nc, in_maps, list(range(N_CORES)))
    out = np.empty((B, INPUT_DIM * E), dtype=np.float32)
    for core in range(N_CORES):
        out[core * B_CORE : (core + 1) * B_CORE] = (
            res.results[core]["outp"].reshape(B_CORE, INPUT_DIM * E)
        )
    return out


# revision 9
# speedup vs baseline: 42.0913x; 33.6251x over previous
"""Trainium2 Bass kernel for vq_codebook (Gaussian-RBF softmax codebook lookup).

reference:
    dist_sq[b,i,k] = (x[b,i] - anchors[k])^2
    w = softmax(-|gamma| * dist_sq, axis=k)
    out[b, i*E+e] = sum_k w[b,i,k] * emb[k,e]

Shapes (hardcoded): x [2048,128] f32, anchors [256] f32, emb [256,64] f32,
gamma scalar f32. Output [2048, 8192] f32.

Strategy: data-parallel over batch across 8 cores (256 batches/core,
M = 256*128 = 32768 scalar elements per core).

Per core:
  z[k,m] = -g*x_m^2 + (2g*a_k)*x_m + (-g*a_k^2)   == -g*(x_m-a_k)^2
  computed by PE as a K=3 matmul: lhsT = Wz [3,128] (two k-halves,
  row-tiled at array rows 0 and 32), rhs = F [3, mchunk] with
  F = [x^2; x; 1]. z lands in PSUM fp32 (exact, no broadcast needed).
  ACT: u = Exp(z) -> bf16 SBUF  (the irreducible compute: 8.4M exps/core)
  PE:  out_psum[m, 0:65] = sum_k u[k,m] * [emb|1][k, e]  (u stationary
       bf16 128-col tiles -> FWL; ones column gives softmax denominator)
  DVE: r = 1/s, out = num * r (per-partition tensor_scalar), DMA out.
"""

import sys

sys.path.insert(0, "/opt/trn_rl_repo")

import numpy as np

import concourse.bass as bass
import concourse.bass2jax as bass2jax
import concourse.mybir as mybir
from concourse.bass_utils import run_bass_kernel_spmd
from concourse.tile import TileContext
from concourse.vector_clock import ScopedClock


def _split_multiwait_bir(bir_json: bytes) -> bytes:
    """This walrus build rejects instructions carrying more than one sync
    wait (codegen setupSyncWait: 'Too many sync wait commands'). Rewrite the
    BIR so any instruction with N>1 waits is preceded by N-1 NoOp carrier
    instructions on the same engine, each holding one wait. Sequencers
    process waits in program order, so semantics are unchanged."""
    import orjson

    d = orjson.loads(bir_json)
    n_split = 0
    for fn in d["functions"]:
        for blk in fn["blocks"]:
            new_insts = []
            dirty = False
            for inst in blk["instructions"]:
                si = inst.get("sync_info")
                waits = (si or {}).get("on_wait") or []
                if len(waits) > 1:
                    dirty = True
                    n_split += 1
                    for j, w in enumerate(waits[:-1]):
                        new_insts.append(
                            {
                                "debug": inst.get("debug", 0),
                                "engine": inst["engine"],
                                "ins": [],
                                "name": f"{inst['name']}-sw{j}",
                                "opcode": "NoOp",
                                "outs": [],
                                "sync_info": {"on_update": [], "on_wait": [w]},
                            }
                        )
                    si["on_wait"] = [waits[-1]]
                new_insts.append(inst)
            if dirty:
                blk["instructions"] = new_insts
    return orjson.dumps(d)


_orig_compile_bir_kernel = bass2jax.compile_bir_kernel


def _patched_compile_bir_kernel(bir_json, tmpdir, neff_name="file.neff"):
    return _orig_compile_bir_kernel(
        _split_multiwait_bir(bir_json), tmpdir, neff_name=neff_name
    )


bass2jax.compile_bir_kernel = _patched_compile_bir_kernel

# problem constants (hardcoded per harness contract)
B, INPUT_DIM, K, E = 2048, 128, 256, 64
N_CORES = 8
B_CORE = B // N_CORES          # 256
M = B_CORE * INPUT_DIM         # 32768 scalar x-elements per core
CHUNK = 512                    # m-elements per pipeline step
N_CHUNKS = M // CHUNK          # 64
KH = K // 2                    # 128 (k-half; k on partitions)

F32 = mybir.dt.float32
BF16 = mybir.dt.bfloat16


class PatchedTileContext(TileContext):
    # This walrus build (CoreV3 setupSyncWait) rejects instructions carrying
    # more than 2 sem waits; the stock Tile tail drain attaches the whole
    # global clock to a single Drain. Split the waits across 1-wait drains.
    def _drain_and_barrier(self, tick_clock, wait_clock):
        drain_inst = self.nc.sync.drain()
        wait_clock.add_sem_waits(
            drain_inst.ins, ScopedClock({None: tick_clock.global_clock})
        )
        si = drain_inst.ins.sync_info
        if si is not None and len(si.on_wait) > 1:
            waits = list(si.on_wait)
            drain_inst.ins.sync_info = mybir.SyncInfo(
                on_wait=waits[:1], on_update=list(si.on_update)
            )
            for w in waits[1:]:
                d2 = self.nc.sync.drain()
                d2.ins.sync_info = mybir.SyncInfo(on_wait=[w], on_update=[])

        self.nc.all_engine_barrier()
        assert self.sems is not None
        popped = self.nc._tile_sem_poison_stack.pop()
        assert popped is self._sem_poison
        self.nc.clear_and_free_semaphores(list(self.sems.allocated().values()))
        self.nc.all_engine_barrier()


def _build_program():
    nc = bass.Bass()
    feats_d = nc.declare_dram_parameter("feats", [3, M], F32, isOutput=False)
    wz_d = nc.declare_dram_parameter("wz", [6, KH], F32, isOutput=False)
    remb_d = nc.declare_dram_parameter("remb", [KH, 2 * (E + 1)], BF16, isOutput=False)
    out_d = nc.declare_dram_parameter("outp", [M, E], F32, isOutput=True)

    EW = E + 1  # 65: emb columns + ones column

    with PatchedTileContext(nc) as tc:
        with (
            tc.tile_pool(name="const", bufs=1) as const_pool,
            tc.tile_pool(name="upool", bufs=4) as upool,
            tc.tile_pool(name="opool", bufs=4) as opool,
            tc.tile_pool(name="rpool", bufs=4) as rpool,
            tc.tile_pool(name="pz", bufs=2, space="PSUM") as pz_pool,
            tc.tile_pool(name="po", bufs=4, space="PSUM") as po_pool,
        ):
            # constants
            feats = const_pool.tile([35, M], F32)
            nc.sync.dma_start(out=feats[0:3, :], in_=feats_d[:, :])
            nc.sync.dma_start(out=feats[32:35, :], in_=feats_d[:, :])
            wz = const_pool.tile([35, KH], F32)
            nc.sync.dma_start(out=wz[0:3, :], in_=wz_d[0:3, :])
            nc.sync.dma_start(out=wz[32:35, :], in_=wz_d[3:6, :])
            remb = const_pool.tile([KH, 2 * EW], BF16)
            nc.sync.dma_start(out=remb[:, :], in_=remb_d[:, :])

            # Host permutes feats columns so that within chunk c, SBUF column
            # j = t*128 + p computes m = c*512 + 4*p + t. Then out_sb
            # [p, t*64+e] is exactly DRAM offset (c*512 + 4p + t)*64 + e:
            # one fully contiguous 128 KiB DMA per chunk.
            out_r = out_d[:, :].rearrange("(c p w) e -> c p (w e)", p=128, w=4)

            for c in range(N_CHUNKS):
                lo = c * CHUNK
                hi = lo + CHUNK

                # z[k, m] for both k-halves, row-tiled (rows 0-2 / 32-34)
                psum_z = pz_pool.tile([128, 2 * CHUNK], F32)
                nc.tensor.matmul(
                    psum_z[:, 0:CHUNK],
                    wz[0:3, :],
                    feats[0:3, lo:hi],
                    start=True,
                    stop=True,
                )
                nc.tensor.matmul(
                    psum_z[:, CHUNK : 2 * CHUNK],
                    wz[32:35, :],
                    feats[32:35, lo:hi],
                    start=True,
                    stop=True,
                )

                # u = exp(z), bf16
                u_sb = upool.tile([128, 2 * CHUNK], BF16)
                nc.scalar.activation(
                    u_sb[:, :], psum_z[:, :], mybir.ActivationFunctionType.Exp
                )

                # out_psum[m, e] = sum_k u[k,m] * remb[k,e], 4 m-tiles of 128
                psum_o = po_pool.tile([128, 4 * EW], F32)
                for t in range(4):
                    nc.tensor.matmul(
                        psum_o[:, t * EW : (t + 1) * EW],
                        u_sb[:, t * 128 : (t + 1) * 128],
                        remb[:, 0:EW],
                        start=True,
                        stop=False,
                    )
                    nc.tensor.matmul(
                        psum_o[:, t * EW : (t + 1) * EW],
                        u_sb[:, CHUNK + t * 128 : CHUNK + (t + 1) * 128],
                        remb[:, EW : 2 * EW],
                        start=False,
                        stop=True,
                    )

                # normalize: r = 1/s (s = ones-column), out = num * r
                po_3d = psum_o.rearrange("p (t w) -> p t w", w=EW)
                r_sb = rpool.tile([128, 4], F32)
                nc.vector.reciprocal(r_sb[:, :], po_3d[:, :, E])
                out_sb = opool.tile([128, 4 * E], F32)
                for t in range(4):
                    nc.vector.tensor_scalar(
                        out_sb[:, t * E : (t + 1) * E],
                        po_3d[:, t, 0:E],
                        r_sb[:, t : t + 1],
                        None,
                        mybir.AluOpType.mult,
                    )
                nc.sync.dma_start(out=out_r[c], in_=out_sb[:, :])

    return nc


_NC_CACHE = None


def _get_program():
    global _NC_CACHE
    if _NC_CACHE is None:
        _NC_CACHE = _build_program()
    return _NC_CACHE


def _feats_perm():
    # column j = c*512 + t*128 + p of the on-device feats tensor must carry
    # element m = c*512 + 4*p + t (see out_r comment in _build_program)
    j = np.arange(M)
    c, r = j // CHUNK, j % CHUNK
    t, p = r // 128, r % 128
    return c * CHUNK + 4 * p + t


_PERM = None


def _prep_core_inputs(x_shard, anchors, embeddings, gamma):
    global _PERM
    if _PERM is None:
        _PERM = _feats_perm()
    g = float(np.abs(np.float32(gamma)))
    xf = np.ascontiguousarray(x_shard, dtype=np.float32).reshape(-1)[_PERM]  # [M]
    feats = np.empty((3, M), dtype=np.float32)
    feats[0] = xf * xf
    feats[1] = xf
    feats[2] = 1.0
    a = np.asarray(anchors, dtype=np.float32)
    wz = np.empty((6, KH), dtype=np.float32)
    for h in range(2):
        ak = a[h * KH : (h + 1) * KH]
        wz[3 * h + 0] = np.float32(-g)
        wz[3 * h + 1] = np.float32(2.0 * g) * ak
        wz[3 * h + 2] = np.float32(-g) * (ak * ak)
    emb = np.asarray(embeddings, dtype=np.float32)
    import ml_dtypes

    EW = E + 1
    remb = np.zeros((KH, 2 * EW), dtype=ml_dtypes.bfloat16)
    for h in range(2):
        remb[:, h * EW : h * EW + E] = emb[h * KH : (h + 1) * KH, :].astype(
            ml_dtypes.bfloat16
        )
        remb[:, h * EW + E] = np.array(1.0, dtype=ml_dtypes.bfloat16)
    return {"feats": feats, "wz": wz, "remb": remb}


def kernel(x, anchors, embeddings, gamma):
    nc = _get_program()
    in_maps = []
    for core in range(N_CORES):
        x_shard = x[core * B_CORE : (core + 1) * B_CORE]
        in_maps.append(_prep_core_inputs(x_shard, anchors, embeddings, gamma))
    res = run_bass_kernel_spmd(nc, in_maps, list(range(N_CORES)))
    out = np.empty((B, INPUT_DIM * E), dtype=np.float32)
    for core in range(N_CORES):
        out[core * B_CORE : (core + 1) * B_CORE] = (
            res.results[core]["outp"].reshape(B_CORE, INPUT_DIM * E)
        )
    return out
